# revision 62
# baseline (speedup 1.0000x reference)
"""Cross-attention kernel for Trainium2, distributed over 8 NeuronCores.

Sharding: batch x head parallel. Cores 0-3 handle batch 0, cores 4-7 batch 1.
Within a team of 4, core r handles heads 4r..4r+3 (channel slice 256r..256r+256).

Per core:
  - KV projection (bf16) for its 256 k-channels + 256 v-channels
  - LayerNorm stats for BOTH q and k: per-row partial (sum, sumsq) over the
    core's 256 channels + one 32KB AllReduce within the team of 4
  - attention for its 4 heads, computed transposed (simT[j,i] = k.q bf16)
    with softmax denominators from an appended ones-column in v (no max
    subtraction: |sim*scale| <= ~6). The exp work is split across THREE
    engines: ACT computes exact exp -> fp8-e4m3 (with a -1 logit bias for
    range), and those slices feed an fp8 DoubleRow attn@V matmul against a
    TWO-TERM fp8 v (v ~= v_hi + v_lo, paired in the DoubleRow weight dim
    with the e-slice broadcast, so v keeps ~7 mantissa bits at half the PE
    cost). DVE and Pool compute a Schraudolph exp (int16(A*sim+B) bit-cast
    as bf16, ~1.8% rms, softmax-scale calibrated) and those slices feed
    plain bf16 attn@V matmuls.
  - attention output produced transposed [256, NQ] in bf16; AllGather within
    team -> [1024, NQ] per 512-column block; each core computes a disjoint
    256-OUTPUT-COLUMN slice of the output projection over all rows.
"""

import numpy as np
import ml_dtypes

import concourse.bass as bass
import concourse.mybir as mybir
import concourse.tile as tile
from concourse import bacc
from concourse.bass_utils import run_bass_kernel_spmd
from concourse.masks import make_identity

B, NQ, NK, D, H, DH = 2, 2048, 2048, 1024, 16, 64
NCORES = 8
TEAM = 4
HPC = 4            # heads per core
DSL = HPC * DH     # 256: per-core channel slice
EPS = 1e-6
SCALE = DH ** -0.5
GROUPS = [[0, 1, 2, 3], [4, 5, 6, 7]]
FP32 = mybir.dt.float32
BF16 = mybir.dt.bfloat16
E4 = mybir.dt.float8e4
I16 = mybir.dt.int16
NT = NQ // 128     # 16 row tiles
KC = D // 128      # 8 contraction chunks
VW = DH + 4        # v columns per head: 64 + ones + 3 pad
OTW = DH + 4       # oT partition rows (= DoubleRow lhsT free/2);
                   # the dual-fp8 ldweights ISA rule needs the pair
                   # step (HPC*VW bytes) to be a multiple of 16

# exp is emitted as exp(scale*sim - BETA); the common e^-BETA factor cancels
# in the softmax ratio and keeps exp(max logit ~5.5) inside e4m3 range (240).
BETA = 1.0
LOG2E = 1.4426950408889634
# Schraudolph trick for the DVE/Pool exp share: I16 = A_SCH*sim + B_SCH,
# bitcast int16 -> bf16 ~= exp(SCALE*sim - BETA). The -7.342 calibration
# centers the piecewise-linear sawtooth AND rescales the mean to match the
# exact-exp slices (the softmax denominator mixes both kinds).
A_SCH = 128.0 * LOG2E * SCALE
B_SCH = 128.0 * (127.0 - BETA * LOG2E) - 7.342

_CACHE: dict = {}
MOCK_COLL = False  # replace collectives with local DMA (for TimelineSim)


def _bcast_ap(t, parts):
    ap = t.ap() if hasattr(t, "ap") and not isinstance(t, bass.AP) else t
    return bass.AP(tensor=ap.tensor, offset=ap.offset,
                   ap=[[0, parts]] + list(ap.ap))


def _pair_bcast(ap3):
    """Insert a stride-0 pair dim after the partition dim: [K, N] view ->
    [K, 2, N] where both pair halves read the same data (used as DoubleRow
    rhs against a (v_hi, v_lo) weight pair)."""
    a = [list(x) for x in ap3.ap]
    return bass.AP(tensor=ap3.tensor, offset=ap3.offset,
                   ap=[a[0], [0, 2]] + a[1:])


def _exp_engine(iblk, h, jp):
    """Which engine computes the exp slice for j-pair jp of head-block
    (iblk, h). 'A' = ACT exact exp -> e4m3 (fp8 DoubleRow AV), 'D'/'P' =
    DVE/Pool Schraudolph -> bf16 (bf16 AV). Block 0 is the software-
    pipelined prologue: keep Pool/DVE free for the stage-A/D work there."""
    return 'A'


def _build():
    nc = bacc.Bacc("TRN2", target_bir_lowering=False, debug=False,
                   num_devices=NCORES)
    # all loads host-pre-arranged for fully contiguous DMA
    x_pre = nc.declare_dram_parameter("x_pre", [128, NT * DSL], BF16,
                                      isOutput=False)
    ctx_pre = nc.declare_dram_parameter("ctx_pre", [NT, 128, KC * 128], BF16,
                                        isOutput=False)
    wkv_pre = nc.declare_dram_parameter("wkv_pre", [KC, 128, 2 * DSL], BF16,
                                        isOutput=False)
    wout_pre = nc.declare_dram_parameter("wout_pre", [128, KC * DSL], BF16,
                                         isOutput=False)
    bout = nc.declare_dram_parameter("bout", [DSL], FP32, isOutput=False)
    # gq/bq/gk/bk packed host-side, pre-transposed per 128-column block:
    # column j = (param j//2, block j%2)
    gparams = nc.declare_dram_parameter("gparams", [128, 8], FP32,
                                        isOutput=False)
    y_full = nc.declare_dram_parameter("y_full", [NQ, DSL], FP32, isOutput=True)

    stats_dram_q = [nc.dram_tensor(f"stats_dram{i}", [128, 16], FP32)
                    for i in range(4)]
    statsr_dram_q = [nc.dram_tensor(f"statsr_dram{i}", [128, 16], FP32)
                     for i in range(4)]
    aoT_blk = [nc.dram_tensor(f"aoT_blk{i}", [DSL, 512], BF16) for i in range(4)]
    # AllGather halves: 'a' = each core's first 128 channels (heads 0-1),
    # 'b' = second 128 (heads 2-3); wout_pre rows are host-permuted to match
    agT_a = [nc.dram_tensor(f"agT_a{i}", [4 * 128, 512], BF16) for i in range(4)]
    agT_b = [nc.dram_tensor(f"agT_b{i}", [4 * 128, 512], BF16) for i in range(4)]

    with tile.TileContext(nc) as tc:
        with (
            tc.tile_pool(name="singles", bufs=1) as singles,
            tc.tile_pool(name="ld", bufs=3) as ld,
            tc.tile_pool(name="work", bufs=3) as work,
            tc.tile_pool(name="psmm", bufs=2, space="PSUM") as psmm,
            tc.tile_pool(name="pssim", bufs=2, space="PSUM") as pssim,
            tc.tile_pool(name="psout", bufs=2, space="PSUM") as psout,
        ):
            # --- persistent sbuf; DMAs ordered so stage A starts ASAP.
            # All input loads go on the (otherwise idle) SP queue with NO
            # semaphore waits: in the cost model a DMA's waits hold the
            # issuing sequencer, so wait-free ordering avoids head-of-line
            # blocking. ctx is loaded fully upfront (no ring). ---
            wkv_sb = singles.tile([128, KC, 2 * DSL], BF16)
            ctx_all = singles.tile([128, NT, KC, 128], BF16, name="ctx_all")

            def _ctx_pair_in(t2):
                return (ctx_pre.ap()[2 * t2:2 * t2 + 2]
                        .rearrange("t p c -> p t c"))

            # x first: the 16 x-square partial stats hold ACT until x lands,
            # and they gate the first stats AllReduce
            x_nat = singles.tile([128, NT, DSL], BF16, name="x_nat")
            nc.sync.dma_start(out=x_nat.rearrange("p t c -> p (t c)"),
                              in_=x_pre.ap())
            gp_sb = singles.tile([128, 8], FP32, name="gp_sb")
            nc.sync.dma_start(out=gp_sb, in_=gparams.ap())
            nc.sync.dma_start(
                out=ctx_all[:, 0:2, :, :].rearrange("p t k c -> p (t k c)"),
                in_=_ctx_pair_in(0))
            nc.sync.dma_start(out=wkv_sb[:, 0, :], in_=wkv_pre.ap()[0])
            nc.sync.dma_start(
                out=wkv_sb[:, 1:KC, :].rearrange("p k c -> p (k c)"),
                in_=wkv_pre.ap()[1:KC].rearrange("k p c -> p k c"))
            for t2 in range(1, NT // 2):
                nc.sync.dma_start(
                    out=ctx_all[:, 2 * t2:2 * t2 + 2, :, :]
                    .rearrange("p t k c -> p (t k c)"),
                    in_=_ctx_pair_in(t2))
            identb = singles.tile([128, 128], BF16)
            make_identity(nc, identb)
            nbeta_sb = singles.tile([128, 1], FP32)
            nc.vector.memset(nbeta_sb, -BETA)
            # magic constant for the fast-rsqrt bit trick (stats math runs
            # entirely on DVE so ACT never loads the Sqrt table set)
            magic_sb = singles.tile([128, 1], mybir.dt.int32)
            nc.vector.memset(magic_sb, 0x5F3759DF)

            gqT = [gp_sb[:, cb:cb + 1] for cb in range(2)]
            bqT = [gp_sb[:, 2 + cb:3 + cb] for cb in range(2)]
            gkT = [gp_sb[:, 4 + cb:5 + cb] for cb in range(2)]
            bkT = [gp_sb[:, 6 + cb:7 + cb] for cb in range(2)]

            k_nat = singles.tile([128, NT, DSL], FP32)
            # v in TWO fp8-e4m3 term PLANES (hi + lo) for the DoubleRow AV:
            # the attn@V matmuls pair adjacent j-tiles in the DoubleRow
            # weight dim, one matmul per plane. The hi ones-column is 1.0
            # and the lo one 0.0 so hi+lo keeps the softmax denominator
            # exact. Layout per plane: [tile][head][DH + ones + pad].
            vh2h = singles.tile([128, NT, HPC * VW], E4, name="vh2h")
            vh2l = singles.tile([128, NT, HPC * VW], E4, name="vh2l")
            for h in range(HPC):
                o2 = h * VW
                nc.vector.memset(vh2h[:, :, o2 + DH:o2 + VW], 0.0)
                nc.vector.memset(vh2h[:, :, o2 + DH:o2 + DH + 1], 1.0)
                nc.vector.memset(vh2l[:, :, o2 + DH:o2 + VW], 0.0)
            qT_sb = [singles.tile([128, NT, 128], BF16, tag=f"qT{cb}",
                                  name=f"qT{cb}") for cb in range(2)]
            kT_sb = [singles.tile([128, NT, 128], BF16, tag=f"kT{cb}",
                                  name=f"kT{cb}") for cb in range(2)]
            aoT_sb = [singles.tile([128, NQ], BF16, tag=f"aoT{cb}",
                                   name=f"aoT{cb}") for cb in range(2)]
            # stats cols: 0=ksum 1=ksumsq 2=qsum 3=qsumsq; QUARTER-granular
            # (4 tiles each) so the AllReduce pipeline overlaps the kv-proj
            # and the first attention exps start as early as possible
            QT = 4
            NQR = NT // QT
            stats_q = [singles.tile([128, QT, 4], FP32, name=f"stats{i}")
                       for i in range(NQR)]
            statsr_q = [singles.tile([128, QT, 4], FP32, name=f"statsr{i}")
                        for i in range(NQR)]
            mean_k = [singles.tile([128, QT], FP32, name=f"mean_k{i}")
                      for i in range(NQR)]
            rstd_k = [singles.tile([128, QT], FP32, name=f"rstd_k{i}")
                      for i in range(NQR)]
            negm_k = [singles.tile([128, QT], FP32, name=f"negm_k{i}")
                      for i in range(NQR)]
            mean_q = [singles.tile([128, QT], FP32, name=f"mean_q{i}")
                      for i in range(NQR)]
            rstd_q = [singles.tile([128, QT], FP32, name=f"rstd_q{i}")
                      for i in range(NQR)]

            # --- stage A: kv-proj + partial LN stats (per tile) ---
            def stage_a(t):
                kv_ps = psmm.tile([128, 2 * DSL], FP32, tag="mm512")
                for kk in range(KC):
                    nc.tensor.matmul(kv_ps, lhsT=ctx_all[:, t, kk, :],
                                     rhs=wkv_sb[:, kk, :],
                                     start=(kk == 0), stop=(kk == KC - 1))
                nc.vector.tensor_copy(k_nat[:, t, :], kv_ps[:, 0:DSL])
                # v: strided copies for all 4 heads (fp8-hi, fp8-lo)
                v_src = kv_ps[:, DSL:2 * DSL].rearrange("p (h c) -> p h c",
                                                        h=HPC)
                vh_t = vh2h[:, t, :].rearrange("p (h c) -> p h c", h=HPC)
                vl_t = vh2l[:, t, :].rearrange("p (h c) -> p h c", h=HPC)
                nc.vector.tensor_copy(vh_t[:, :, 0:DH], v_src)
                nc.vector.tensor_tensor(out=vl_t[:, :, 0:DH],
                                        in0=v_src, in1=vh_t[:, :, 0:DH],
                                        op=mybir.AluOpType.subtract)
                # partial k stats: sum on DVE, sum-of-squares on ACT
                qr, c = t // QT, t % QT
                st = stats_q[qr]
                nc.vector.reduce_sum(out=st[:, c, 0:1], in_=k_nat[:, t, :],
                                     axis=mybir.AxisListType.X)
                scr = work.tile([128, DSL], BF16, tag="sqscr", bufs=2)
                nc.scalar.activation(scr, kv_ps[:, 0:DSL],
                                     mybir.ActivationFunctionType.Square,
                                     accum_out=st[:, c, 1:2])

            def x_stats(t):
                # q stats depend only on x: run for ALL tiles before the kv
                # loop so the stats AllReduce is gated by the k accumulations
                # alone (ACT is otherwise idle at kernel start)
                qr, c = t // QT, t % QT
                st = stats_q[qr]
                nc.vector.reduce_sum(out=st[:, c, 2:3], in_=x_nat[:, t, :],
                                     axis=mybir.AxisListType.X)
                scr2 = work.tile([128, DSL], BF16, tag="sqscr2", bufs=2)
                nc.scalar.activation(scr2, x_nat[:, t, :],
                                     mybir.ActivationFunctionType.Square,
                                     accum_out=st[:, c, 3:4])

            def issue_allreduce(qr):
                nc.sync.dma_start(
                    out=stats_dram_q[qr][:, :],
                    in_=stats_q[qr].rearrange("p t s -> p (t s)"))
                if MOCK_COLL:
                    nc.sync.dma_start(out=statsr_dram_q[qr][:, :],
                                      in_=stats_dram_q[qr][:, :])
                else:
                    nc.gpsimd.collective_compute(
                        "AllReduce", mybir.AluOpType.add,
                        replica_groups=GROUPS,
                        ins=[stats_dram_q[qr].ap().opt()],
                        outs=[statsr_dram_q[qr].ap().opt()])
                nc.sync.dma_start(
                    out=statsr_q[qr].rearrange("p t s -> p (t s)"),
                    in_=statsr_dram_q[qr][:, :])

            def stats_math(qr):
                for mean_t, rstd_t, negm_t, c0 in (
                        (mean_k[qr], rstd_k[qr], negm_k[qr], 0),
                        (mean_q[qr], rstd_q[qr], None, 2)):
                    nc.vector.tensor_scalar_mul(
                        mean_t, in0=statsr_q[qr][:, :, c0], scalar1=1.0 / D)
                    nc.vector.tensor_scalar_mul(
                        rstd_t, in0=statsr_q[qr][:, :, c0 + 1], scalar1=1.0 / D)
                    m2 = work.tile([128, QT], FP32, tag="m2", bufs=2)
                    nc.vector.tensor_mul(m2, mean_t, mean_t)
                    nc.vector.tensor_sub(rstd_t, rstd_t, m2)
                    # rstd = rsqrt(var + eps), entirely on DVE: bit-trick
                    # seed + 2 Newton iterations (~1e-5 rel err; var is O(1))
                    nc.vector.tensor_scalar_add(rstd_t, in0=rstd_t,
                                                scalar1=EPS)
                    jt = work.tile([128, QT], mybir.dt.int32, tag="rsj",
                                   bufs=2)
                    nc.vector.tensor_single_scalar(
                        out=jt, in_=rstd_t.bitcast(mybir.dt.int32),
                        scalar=1, op=mybir.AluOpType.logical_shift_right)
                    yt = work.tile([128, QT], mybir.dt.int32, tag="rsy",
                                   bufs=2)
                    mg = magic_sb[:, 0:1]
                    mg_b = bass.AP(tensor=mg.tensor, offset=mg.offset,
                                   ap=[list(mg.ap[0]), [0, QT]])
                    nc.vector.tensor_tensor(out=yt, in0=mg_b, in1=jt,
                                            op=mybir.AluOpType.subtract)
                    y_f = yt.bitcast(FP32)
                    at = work.tile([128, QT], FP32, tag="rsa", bufs=2)
                    for it in range(2):
                        nc.vector.tensor_mul(at, rstd_t, y_f)
                        nc.vector.tensor_mul(at, at, y_f)
                        nc.vector.tensor_scalar(
                            out=at, in0=at, scalar1=-0.5, scalar2=1.5,
                            op0=mybir.AluOpType.mult,
                            op1=mybir.AluOpType.add)
                        if it == 0:
                            nc.vector.tensor_mul(yt.bitcast(FP32), y_f, at)
                        else:
                            nc.vector.tensor_mul(rstd_t, y_f, at)
                    if negm_t is not None:
                        nc.vector.tensor_mul(negm_t, mean_t, rstd_t)
                        nc.vector.tensor_scalar_mul(negm_t, in0=negm_t,
                                                    scalar1=-1.0)

            def stage_d(t, mode="pool"):
                # k LN apply on Pool, q on DVE: NEVER on ACT (the k-squares
                # gating the stats AllReduce queue behind them there), and
                # never both on one engine (stage-A Pool is busy with the v
                # copies).
                qr, c = t // QT, t % QT
                k_ln = work.tile([128, DSL], BF16, tag="kln", bufs=4)
                q_ln = work.tile([128, DSL], BF16, tag="qln", bufs=4)
                nc.gpsimd.tensor_scalar(out=k_ln,
                                        in0=k_nat[:, t, :],
                                        scalar1=mean_k[qr][:, c:c + 1],
                                        scalar2=rstd_k[qr][:, c:c + 1],
                                        op0=mybir.AluOpType.subtract,
                                        op1=mybir.AluOpType.mult)
                nc.vector.tensor_scalar(out=q_ln, in0=x_nat[:, t, :],
                                        scalar1=mean_q[qr][:, c:c + 1],
                                        scalar2=rstd_q[qr][:, c:c + 1],
                                        op0=mybir.AluOpType.subtract,
                                        op1=mybir.AluOpType.mult)
                for cb in range(2):
                    tp_ps = pssim.tile([128, 128], BF16, tag="sim")
                    nc.tensor.transpose(tp_ps,
                                        k_ln[:, 128 * cb:128 * (cb + 1)],
                                        identb)
                    nc.vector.tensor_scalar(out=kT_sb[cb][:, t, :],
                                            in0=tp_ps,
                                            scalar1=gkT[cb], scalar2=bkT[cb],
                                            op0=mybir.AluOpType.mult,
                                            op1=mybir.AluOpType.add)
                    tq_ps = pssim.tile([128, 128], BF16, tag="sim")
                    nc.tensor.transpose(tq_ps,
                                        q_ln[:, 128 * cb:128 * (cb + 1)],
                                        identb)
                    nc.vector.tensor_scalar(out=qT_sb[cb][:, t, :],
                                            in0=tq_ps,
                                            scalar1=gqT[cb],
                                            scalar2=bqT[cb],
                                            op0=mybir.AluOpType.mult,
                                            op1=mybir.AluOpType.add)

            # x-only stats for ALL tiles first (ACT is idle early)
            for t in range(NT):
                x_stats(t)

            wout_sb = singles.tile([128, KC, DSL], BF16)
            nc.sync.dma_start(out=wout_sb.rearrange("p k n -> p (k n)"),
                              in_=wout_pre.ap())
            bout_b = singles.tile([128, DSL], FP32)
            nc.sync.dma_start(out=bout_b, in_=_bcast_ap(bout, 128))

            # --- stage F+G+H fused: attention -> per-block AllGather ->
            # out-projection (256 output columns), pipelined over 512-column
            # blocks of NQ ---
            def head_attn(iblk, h, jps, oT_ps, extras=None):
                cb, hh = h // 2, h % 2
                khT = kT_sb[cb][64 * hh:64 * (hh + 1), :, :]
                qhT = qT_sb[cb][64 * hh:64 * (hh + 1), :, :]
                o2 = h * VW
                for jp in jps:
                    s_ps = pssim.tile([128, 2, 512], FP32, tag="sim")
                    for jj in range(2):
                        j = 2 * jp + jj
                        nc.tensor.matmul(
                            s_ps[:, jj, :], lhsT=khT[:, j, :],
                            rhs=qhT[:, 4 * iblk:4 * (iblk + 1), :],
                            start=True, stop=True)
                    e16 = work.tile([128, 2, 512], I16, tag="exp", bufs=6)
                    eng = _exp_engine(iblk, h, jp)
                    if eng == 'A':
                        e4_v = e16.bitcast(E4)[:, :, 0:512]
                        nc.scalar.activation(
                            e4_v, s_ps[:, :, :],
                            mybir.ActivationFunctionType.Exp,
                            scale=SCALE, bias=nbeta_sb)
                        # two DoubleRow matmuls per slice (hi + lo v plane),
                        # each contracting BOTH j-tiles of the pair
                        nc.tensor.matmul(
                            oT_ps,
                            lhsT=vh2h[:, 2 * jp:2 * jp + 2, o2:o2 + VW],
                            rhs=e4_v,
                            start=(jp == 0), stop=False,
                            perf_mode=mybir.MatmulPerfMode.DoubleRow)
                        nc.tensor.matmul(
                            oT_ps,
                            lhsT=vh2l[:, 2 * jp:2 * jp + 2, o2:o2 + VW],
                            rhs=e4_v,
                            start=False, stop=(jp == NT // 2 - 1),
                            perf_mode=mybir.MatmulPerfMode.DoubleRow)
                    else:
                        tse = (nc.vector if eng == 'D' else
                               nc.gpsimd).tensor_scalar
                        with tc.high_priority():
                            tse(out=e16[:, :, :], in0=s_ps[:, :, :],
                                scalar1=A_SCH, scalar2=B_SCH,
                                op0=mybir.AluOpType.mult,
                                op1=mybir.AluOpType.add)
                        ebf = e16.bitcast(BF16)
                        for jj in range(2):
                            j = 2 * jp + jj
                            nc.tensor.matmul(
                                oT_ps, lhsT=vhb[:, j, ob:ob + VW],
                                rhs=ebf[:, jj, :],
                                start=(j == 0), stop=(j == NT - 1))
                    if extras:
                        step = extras.popleft()
                        if step is not None:
                            # negative offset = lower scheduler priority:
                            # out-projection work must never displace the
                            # QK/exp/AV chain
                            with tc.high_priority(offset=-100000):
                                step()

            def head_norm(iblk, h, oT_ps):
                # normalize: row DH of oT_ps holds the softmax denominators;
                # invert on DVE, broadcast AND multiply on Pool (keeping the
                # mul off DVE avoids head-of-line blocking the Schraudolph
                # slices queued behind it on DVE)
                cb, hh = h // 2, h % 2
                cs = work.tile([1, 512], FP32, tag="cs", bufs=2)
                nc.vector.reciprocal(cs, oT_ps[DH:DH + 1, :])
                csb = work.tile([64, 512], FP32, tag="csb", bufs=2)
                nc.gpsimd.partition_broadcast(csb, cs)
                nc.vector.tensor_mul(
                    aoT_sb[cb][64 * hh:64 * (hh + 1),
                               512 * iblk:512 * (iblk + 1)],
                    oT_ps[0:DH, :], csb)

            def gather_half(iblk, half):
                # store this core's 128-channel half of the block (issued
                # from DVE right after the producing mul, so the DMA's waits
                # are already satisfied), then AllGather it within the team
                # (output row-order is the team-concat of these halves;
                # wout_pre rows match it)
                nc.sync.dma_start(
                    out=aoT_blk[iblk][128 * half:128 * (half + 1), :],
                    in_=aoT_sb[half][:, 512 * iblk:512 * (iblk + 1)])
                ag_t = (agT_a, agT_b)[half][iblk]
                src_ap = aoT_blk[iblk].ap()[128 * half:128 * (half + 1), :]
                if MOCK_COLL:
                    nc.sync.dma_start(out=ag_t.ap()[0:128, :], in_=src_ap)
                else:
                    nc.gpsimd.collective_compute(
                        "AllGather", mybir.AluOpType.bypass,
                        replica_groups=GROUPS,
                        ins=[src_ap.opt()], outs=[ag_t.ap().opt()])

            # out-projection work for one block half, cut into per-j-pair
            # closures (2 matmuls each) fed into head_attn's `extras` so the
            # PE queue never sees a long projection burst
            def outproj_steps(iblk, half, yacc_cell):
                steps = []
                ag_t = (agT_a, agT_b)[half][iblk]
                ag_r = ag_t.ap().rearrange("(k p) n -> p k n", p=128)
                ld_cell = {}

                def load():
                    ag_sb = ld.tile([128, 4, 512], BF16, tag="ag", bufs=2,
                                    name="ag_sb")
                    nc.sync.dma_start(out=ag_sb, in_=ag_r)
                    ld_cell["t"] = ag_sb
                    if half == 0:
                        yacc_cell["t"] = work.tile([128, 4, DSL], FP32,
                                                   tag="yacc", bufs=2,
                                                   name="yacc")
                    else:
                        yacc_cell["y"] = work.tile([128, 4, DSL], FP32,
                                                   tag="yout", bufs=2,
                                                   name="yout")
                steps.append(load)
                # the AllGather result lands several slices after the load
                # is issued: stagger the matmul steps so a not-yet-satisfied
                # wait never sits at the head of the PE queue blocking QKs
                steps.extend([None, None, None])
                for sub in range(4):

                    def mm(sub=sub):
                        # one short PE burst + immediate add: y_ps shares
                        # the sim ring, so it must free quickly
                        y_ps = psmm.tile([128, DSL], FP32, tag="mm512",
                                         name="y_ps")
                        for kk in range(4):
                            nc.tensor.matmul(
                                y_ps,
                                lhsT=ld_cell["t"][:, kk,
                                                  128 * sub:128 * (sub + 1)],
                                rhs=wout_sb[:, 4 * half + kk, :],
                                start=(kk == 0), stop=(kk == 3))
                        yacc = yacc_cell["t"]
                        if half == 0:
                            nc.vector.tensor_add(yacc[:, sub, :], y_ps,
                                                 bout_b)
                        else:
                            yout = yacc_cell["y"]
                            nc.vector.tensor_add(yout[:, sub, :], y_ps,
                                                 yacc[:, sub, :])
                            if sub == 3:
                                # one batched row-block store per 512 rows
                                nc.sync.dma_start(
                                    out=(y_full.ap()
                                         [512 * iblk:512 * (iblk + 1), :]
                                         .rearrange("(s p) c -> p s c",
                                                    p=128)),
                                    in_=yout.rearrange("p s c -> p (s c)"))
                    steps.append(mm)
                return steps

            from collections import deque
            extras = deque()
            yacc_cells = [dict() for _ in range(4)]

            # --- software-pipelined prologue: the kv-proj / stats-AllReduce
            # / LayerNorm pipeline for later tiles is interleaved with the
            # first two head-blocks' attention (which only needs earlier
            # tiles), so the exp stream starts as soon as the first quarter
            # of k is normalized ---
            def hot(fn, *a):
                # stats/LN chain is the prologue critical path: schedule its
                # ops as soon as their deps resolve, ahead of bulk stage-A
                # copy work queued earlier on the same engines
                with tc.high_priority():
                    fn(*a)

            for t in range(0, QT):
                stage_a(t)
            hot(issue_allreduce, 0)
            hot(stats_math, 0)
            stage_a(4); hot(stage_d, 0)
            stage_a(5); hot(stage_d, 1)
            stage_a(6); hot(stage_d, 2)
            stage_a(7); hot(stage_d, 3)
            hot(issue_allreduce, 1)
            hot(stats_math, 1)
            oT00 = psout.tile([OTW, 512], FP32, tag="oT")
            head_attn(0, 0, [0], oT00)
            stage_a(8); hot(stage_d, 4)
            stage_a(9); hot(stage_d, 5)
            head_attn(0, 0, [1], oT00)
            stage_a(10); hot(stage_d, 6)
            stage_a(11); hot(stage_d, 7)
            hot(issue_allreduce, 2)
            hot(stats_math, 2)
            head_attn(0, 0, [2], oT00)
            oT01 = psout.tile([OTW, 512], FP32, tag="oT")
            head_attn(0, 1, [0], oT01)
            stage_a(12); hot(stage_d, 8)
            stage_a(13); hot(stage_d, 9)
            head_attn(0, 0, [3], oT00)
            head_attn(0, 1, [1], oT01)
            stage_a(14); hot(stage_d, 10)
            stage_a(15); hot(stage_d, 11)
            hot(issue_allreduce, 3)
            hot(stats_math, 3)
            head_attn(0, 1, [2, 3], oT01)
            hot(stage_d, 12); hot(stage_d, 13)
            head_attn(0, 0, [4], oT00)
            hot(stage_d, 14); hot(stage_d, 15)
            head_attn(0, 0, [5, 6, 7], oT00)
            head_norm(0, 0, oT00)
            head_attn(0, 1, [4, 5, 6, 7], oT01)
            head_norm(0, 1, oT01)
            gather_half(0, 0)
            extras.extend(outproj_steps(0, 0, yacc_cells[0]))
            for h in (2, 3):
                oT_ps = psout.tile([OTW, 512], FP32, tag="oT")
                head_attn(0, h, range(NT // 2), oT_ps, extras)
                head_norm(0, h, oT_ps)
            gather_half(0, 1)
            extras.extend(outproj_steps(0, 1, yacc_cells[0]))

            for iblk in range(1, 4):
                for h in range(HPC):
                    oT_ps = psout.tile([OTW, 512], FP32, tag="oT")
                    head_attn(iblk, h, range(NT // 2), oT_ps, extras)
                    head_norm(iblk, h, oT_ps)
                    if h == 1:
                        gather_half(iblk, 0)
                        extras.extend(outproj_steps(iblk, 0,
                                                    yacc_cells[iblk]))
                gather_half(iblk, 1)
                extras.extend(outproj_steps(iblk, 1, yacc_cells[iblk]))
            while extras:
                step = extras.popleft()
                if step is not None:
                    step()

    nc.finalize()
    return nc


def _host_prep(x, context, gq, bq, gk, bk, W_kv, W_out, b_out):
    bf = ml_dtypes.bfloat16
    Wk, Wv = W_kv[:D], W_kv[D:]
    in_maps = []
    for c in range(NCORES):
        b, r = c // TEAM, c % TEAM
        sl = slice(DSL * r, DSL * (r + 1))
        wkv_c = (np.concatenate([Wk[sl], Wv[sl]], axis=0).T.astype(bf)
                 .reshape(KC, 128, 2 * DSL))
        ctx_c = (context[b].astype(bf).reshape(NT, 128, KC, 128)
                 .transpose(0, 3, 2, 1).reshape(NT, 128, KC * 128))
        x_c = (x[b][:, sl].astype(bf).reshape(NT, 128, DSL)
               .transpose(1, 0, 2).reshape(128, NT * DSL))
        # wout rows permuted to the AllGather-half concat order:
        # chunks 0-3 = team channels 256c..256c+128 (half a, heads 0-1 of
        # each core), chunks 4-7 = 256c+128..256c+256 (half b)
        wt = W_out.T[:, sl].astype(bf).reshape(TEAM, 2, 128, DSL)
        wout_c = (np.concatenate([wt[:, 0], wt[:, 1]], axis=0)
                  .transpose(1, 0, 2).reshape(128, KC * DSL))
        # packed LN params, pre-transposed: col j = (param j//2, block j%2)
        gp = np.stack([gq[sl][:128], gq[sl][128:],
                       bq[sl][:128], bq[sl][128:],
                       gk[sl][:128], gk[sl][128:],
                       bk[sl][:128], bk[sl][128:]], axis=1)
        in_maps.append({
            "x_pre": np.ascontiguousarray(x_c),
            "ctx_pre": np.ascontiguousarray(ctx_c),
            "wkv_pre": np.ascontiguousarray(wkv_c),
            "wout_pre": np.ascontiguousarray(wout_c),
            "bout": np.ascontiguousarray(b_out[sl]),
            "gparams": np.ascontiguousarray(gp.astype(np.float32)),
        })
    return in_maps


def kernel(x, context, gq, bq, gk, bk, W_kv, W_out, b_out):
    x = np.asarray(x, dtype=np.float32)
    context = np.asarray(context, dtype=np.float32)
    gq = np.asarray(gq, dtype=np.float32)
    bq = np.asarray(bq, dtype=np.float32)
    gk = np.asarray(gk, dtype=np.float32)
    bk = np.asarray(bk, dtype=np.float32)
    W_kv = np.asarray(W_kv, dtype=np.float32)
    W_out = np.asarray(W_out, dtype=np.float32)
    b_out = np.asarray(b_out, dtype=np.float32)

    if "nc" not in _CACHE:
        _CACHE["nc"] = _build()
    nc = _CACHE["nc"]

    in_maps = _host_prep(x, context, gq, bq, gk, bk, W_kv, W_out, b_out)
    _CACHE["in_maps"] = in_maps
    try:
        res = run_bass_kernel_spmd(nc, in_maps, list(range(NCORES))).results
    except Exception:
        # transient runtime failures (device wedged from a prior run) --
        # one retry typically succeeds
        res = run_bass_kernel_spmd(nc, in_maps, list(range(NCORES))).results
    y = np.empty((B, NQ, D), dtype=np.float32)
    for c in range(NCORES):
        b, r = c // TEAM, c % TEAM
        y[b, :, DSL * r:DSL * (r + 1)] = res[c]["y_full"]
    return y
